# revision 52
# baseline (speedup 1.0000x reference)
"""Multi-head attention Trainium2 kernel (B=4, S=2048, D=1024, H=16, causal).

Sharding: 8 cores = 4 batches x 2 head-groups (8 heads each, tensor-parallel
over the QKV/out projection weights along the head dimension).

Per-core strategy (v2 — fp8 DoubleRow projections + f16 attention):
  - Projections run as fp8e4m3 DoubleRow matmuls (0.5 PE cycles/row, two
    128-deep k-chunks per instruction) with a 3-term error correction:
      W'x ~= Wh.xh + (Wh/32).(32 xl) + (W' - Wh).xh
    where W' = 512W (host-scaled so both Wh and the residual W'-Wh sit in
    e4m3's normal range), xh = q8(x), xl = x - xh.  All three terms share
    one PSUM accumulation (scale 512); the DVE bias-add folds the 1/512
    back in.  All quantization happens on the host, shipped as fp8 arrays
    (2 bytes/activation value vs 4 for f32 -> half the DMA).
  - Scores in f16 (fp8 scores fail the 2e-2 gate), exp on ACT with
    scale=1/8 and bias=-2 folded in (denominator self-normalizes), E in
    f16 (max logit 10.31 -> e^8.3 fits f16 comfortably).
  - ctx matmuls are operand-swapped: out [128 q, 65] (cost-model charges
    output free size only -> 65 rows instead of 512), accumulated over
    k-tiles in PSUM; V carries a ones column producing the denominator.
  - normalize on DVE (per-partition reciprocal scalars, no partition
    broadcast needed), then a DMA transpose re-orients ctx to d-major in
    ctxT for the out-projection.
  - Projections for s-block ts+1 and out-projections are injected as PE
    filler inside the ACT-paced attention of q-block qb=ts.
"""

import numpy as np
import ml_dtypes

import concourse.bacc as bacc
import concourse.mybir as mybir
import concourse.tile as tile
from concourse.bass_utils import run_bass_kernel_spmd

B, S, D, H = 4, 2048, 1024, 16
DK = D // H          # 64
N_CORES = 8
O = 512              # head dims per core (8 heads x 64)
HPC = 8              # heads per core
SB = 512             # s-block for projections
QB = 512             # q-block for attention
KT = 128             # k tile
F32 = mybir.dt.float32
F16 = mybir.dt.float16
F8 = mybir.dt.float8e4
DR = mybir.MatmulPerfMode.DoubleRow
NF8 = ml_dtypes.float8_e4m3

_CACHE = {}


def _build(s=S):
    nc = bacc.Bacc("TRN2", target_bir_lowering=False, debug=False,
                   num_devices=N_CORES)
    n_ts = s // SB
    n_qb = s // QB
    n_sc = s // 128

    # fp8 activation arrays [D, s]: hi, lo*32
    xd = {}
    for t in ("q", "k", "v"):
        for a in ("h", "l"):
            xd[t + a] = nc.declare_dram_parameter(f"x{t}{a}", [D, s], F8,
                                                  isOutput=False)
    # fp8 weight arrays [D, O]: hi, hi/32, lo*32
    wd = {}
    for t in ("q", "k", "v"):
        for a in ("h", "3", "l"):
            wd[t + a] = nc.declare_dram_parameter(f"w{t}{a}", [D, O], F8,
                                                  isOutput=False)
    bqd = nc.declare_dram_parameter("bq", [O], F32, isOutput=False)
    bkd = nc.declare_dram_parameter("bk", [O], F32, isOutput=False)
    bvb = nc.declare_dram_parameter("bv_bc", [128, O], F32, isOutput=False)
    wod = nc.declare_dram_parameter("woT", [O, D], F16, isOutput=False)
    maskd = nc.declare_dram_parameter("masks", [KT, KT], F16, isOutput=False)
    onesd = nc.declare_dram_parameter("ones8", [128, HPC], F16,
                                      isOutput=False)
    outd = nc.declare_dram_parameter("out", [s, D], F16, isOutput=True)

    with tile.TileContext(nc) as tc:
        with tc.tile_pool(name="res", bufs=1) as res, \
             tc.tile_pool(name="wpool", bufs=1) as wpool, \
             tc.tile_pool(name="xpool", bufs=2) as xpool, \
             tc.tile_pool(name="epool", bufs=8) as epool, \
             tc.tile_pool(name="npool", bufs=3) as npool, \
             tc.tile_pool(name="outpool", bufs=2) as outpool:
            psum = tc.alloc_tile_pool(name="psum", bufs=2, space="PSUM")

            qhT = [res.tile([128, s], F16, tag=f"qhT{j}", name=f"qhT{j}")
                   for j in range(4)]
            khT = [res.tile([128, s], F16, tag=f"khT{j}", name=f"khT{j}")
                   for j in range(4)]
            vh = [res.tile([128, HPC, DK + 1], F16, tag=f"vh{i}",
                           name=f"vh{i}") for i in range(n_sc)]
            ctxT = [res.tile([128, s], F16, tag=f"ctxT{j}", name=f"ctxT{j}")
                    for j in range(4)]
            bq_t = res.tile([128, O // 128], F32, tag="bq_t", name="bq_t")
            bk_t = res.tile([128, O // 128], F32, tag="bk_t", name="bk_t")
            bv_t = res.tile([128, O], F32, tag="bv_t", name="bv_t")
            masks = res.tile([128, KT], F16, tag="masks", name="masks")
            ones_t = res.tile([128, HPC], F16, tag="ones_t", name="ones_t")
            wo_sb = [res.tile([128, D], F16, tag=f"wo{jw}", name=f"wo{jw}")
                     for jw in range(4)]
            bias2 = res.tile([128, 1], F32, tag="bias2", name="bias2")

            # weight tiles [128, 8, O] fp8 (d-chunk as free dim)
            wt = {}
            for t in ("q", "k", "v"):
                for a in ("h", "3", "l"):
                    wt[t + a] = wpool.tile([128, 8, O], F8, tag=f"w{t}{a}",
                                           name=f"w{t}{a}")

            def load_w(t):
                for a in ("h", "3", "l"):
                    nc.sync.dma_start(
                        wt[t + a][:],
                        wd[t + a].ap().rearrange("(a p) m -> p a m",
                                                 p=128))

            def load_const():
                nc.sync.dma_start(
                    bq_t[:], bqd.ap().rearrange("(m p) -> p m", p=128))
                nc.sync.dma_start(
                    bk_t[:], bkd.ap().rearrange("(m p) -> p m", p=128))
                nc.sync.dma_start(bv_t[:], bvb[:, :])
                nc.sync.dma_start(masks[:], maskd[:, :])
                nc.sync.dma_start(ones_t[:], onesd[:, :])

            def load_wo():
                for jw in range(4):
                    nc.sync.dma_start(wo_sb[jw][:],
                                      wod[jw * 128:(jw + 1) * 128, :])

            xt = {}   # (ts, tensor, arr) -> tile

            def load_x(ts, tensors=("q", "k", "v"), chunked=False):
                for t in tensors:
                    for a in ("h", "l"):
                        tl = xpool.tile([128, 8, SB], F8, tag=f"x{t}{a}",
                                        name=f"x{t}{a}{ts}")
                        src_r = xd[t + a].ap().rearrange(
                            "(a p) s -> p a s", p=128)
                        eng = nc.sync
                        if chunked:
                            for c4 in range(4):
                                ssl = slice(ts * SB + c4 * 128,
                                            ts * SB + (c4 + 1) * 128)
                                eng.dma_start(
                                    tl[:, :, c4 * 128:(c4 + 1) * 128],
                                    src_r[:, :, ssl])
                        else:
                            ssl = slice(ts * SB, (ts + 1) * SB)
                            eng.dma_start(tl[:], src_r[:, :, ssl])
                        xt[(ts, t, a)] = tl

            def proj_qk_unit(t, m, ts, hf=None):
                """One m-tile of the q or k projection for s-block ts."""
                xh, xl = xt[(ts, t, "h")], xt[(ts, t, "l")]
                msl = slice(m * 128, (m + 1) * 128)
                ps = psum.tile([128, SB], F32, tag="fo", name=f"ps{t}")
                for c in range(4):
                    cs = slice(2 * c, 2 * c + 2)
                    nc.tensor.matmul(ps[:], wt[t + "h"][:, cs, msl],
                                     xh[:, cs, :], start=(c == 0),
                                     stop=False, perf_mode=DR)
                for c in range(4):
                    cs = slice(2 * c, 2 * c + 2)
                    nc.tensor.matmul(ps[:], wt[t + "3"][:, cs, msl],
                                     xl[:, cs, :], start=False, stop=False,
                                     perf_mode=DR)
                for c in range(4):
                    cs = slice(2 * c, 2 * c + 2)
                    nc.tensor.matmul(ps[:], wt[t + "l"][:, cs, msl],
                                     xh[:, cs, :], start=False,
                                     stop=(c == 3), perf_mode=DR)
                dstT = qhT if t == "q" else khT
                bias = bq_t if t == "q" else bk_t
                ssl = slice(ts * SB, (ts + 1) * SB)
                nc.vector.scalar_tensor_tensor(
                    dstT[m][:, ssl], ps[:], 1.0 / 512.0,
                    bias[:, m:m + 1].broadcast_to([128, SB]),
                    op0=mybir.AluOpType.mult, op1=mybir.AluOpType.add)

            def proj_v_unit(sc, ts):
                """One 128-row s-chunk of the v projection for s-block ts."""
                xh, xl = xt[(ts, "v", "h")], xt[(ts, "v", "l")]
                scl = slice(sc * 128, (sc + 1) * 128)
                si = ts * (SB // 128) + sc
                ps = psum.tile([128, O], F32, tag="fo", name="psv")
                for c in range(4):
                    cs = slice(2 * c, 2 * c + 2)
                    nc.tensor.matmul(ps[:], xh[:, cs, scl],
                                     wt["vh"][:, cs, :], start=(c == 0),
                                     stop=False, perf_mode=DR)
                for c in range(4):
                    cs = slice(2 * c, 2 * c + 2)
                    nc.tensor.matmul(ps[:], xl[:, cs, scl],
                                     wt["v3"][:, cs, :], start=False,
                                     stop=False, perf_mode=DR)
                for c in range(4):
                    cs = slice(2 * c, 2 * c + 2)
                    nc.tensor.matmul(ps[:], xh[:, cs, scl],
                                     wt["vl"][:, cs, :], start=False,
                                     stop=(c == 3), perf_mode=DR)
                nc.vector.scalar_tensor_tensor(
                    vh[si][:, :, 0:DK],
                    ps[:].rearrange("p (h e) -> p h e", e=DK), 1.0 / 512.0,
                    bv_t[:].rearrange("p (h e) -> p h e", e=DK),
                    op0=mybir.AluOpType.mult, op1=mybir.AluOpType.add)
                nc.vector.tensor_copy(vh[si][:, :, DK], ones_t[:])

            def proj_units(ts):
                units = []
                for t in ("q", "k"):
                    for m in range(4):
                        units.append(lambda t=t, m=m, ts=ts:
                                     proj_qk_unit(t, m, ts))
                for sc in range(SB // 128):
                    units.append(lambda sc=sc, ts=ts: proj_v_unit(sc, ts))
                return units

            def outproj_unit(sc):
                ot = outpool.tile([128, D], F16, tag="out_t", name="ot")
                for oc in range(2):
                    osl = slice(oc * 512, (oc + 1) * 512)
                    po = psum.tile([128, 512], F32, tag="fo", name="po")
                    for jw in range(4):
                        nc.tensor.matmul(
                            po[:], ctxT[jw][:, sc * 128:(sc + 1) * 128],
                            wo_sb[jw][:, osl], start=(jw == 0),
                            stop=(jw == 3))
                    nc.vector.tensor_copy(ot[:, osl], po[:])
                nc.sync.dma_start(outd[sc * 128:(sc + 1) * 128, :], ot[:])


            def sc_exp(qb, j, t, e01s):
                nt = (qb + 1) * (QB // KT)
                ksl = slice(t * KT, (t + 1) * KT)
                jj = t - (QB // KT) * qb
                lo = jj * KT if jj > 0 else 0
                qn = slice(qb * QB + lo, (qb + 1) * QB)
                s01 = psum.tile([128, 2, QB], F32, tag="sc", name="s01")
                nc.tensor.matmul(
                    s01[:, 0, lo:], khT[j][0:64, ksl],
                    qhT[j][0:64, qn], start=True, stop=True)
                nc.tensor.matmul(
                    s01[:, 1, lo:], khT[j][64:128, ksl],
                    qhT[j][64:128, qn], start=True, stop=True,
                    tile_position=(64, 0))
                e01 = epool.tile([128, 2, QB], F16, tag="e01", name="e01")
                nc.scalar.activation(
                    e01[:, :, lo:], s01[:, :, lo:],
                    mybir.ActivationFunctionType.Exp,
                    bias=bias2[:], scale=0.125)
                if jj >= 0:
                    nc.vector.tensor_mul(
                        e01[:, :, lo:lo + KT],
                        e01[:, :, lo:lo + KT],
                        masks[:].unsqueeze(1).broadcast_to([128, 2, KT]))
                e01s[(j, t)] = e01

            def ctx_unit(qb, j, t, e01s, cp):
                jj = t - (QB // KT) * qb
                e01 = e01s[(j, t)]
                for qc in range(max(jj, 0), 4):
                    last = 4 * qb + qc
                    qcs = slice(qc * KT, (qc + 1) * KT)
                    for h in range(2):
                        nc.tensor.matmul(
                            cp[qc // 2][:, qc % 2, h, :],
                            e01[:, h, qcs], vh[t][:, 2 * j + h, :],
                            start=False, stop=False,
                            skip_group_check=True)

            def norm_unit(qb, j, cp):
                csw = npool.tile([128, 4, 2, DK], F16, tag="csw",
                                 name="csw")
                for i in range(2):
                    rec = npool.tile([128, 2, 2, 1], F32, tag=f"rec{i}",
                                     name=f"rec{i}")
                    nc.vector.reciprocal(rec[:],
                                         cp[i][:, :, :, DK:DK + 1])
                    nc.vector.tensor_mul(
                        csw[:, 2 * i:2 * i + 2, :, :],
                        cp[i][:, :, :, 0:DK],
                        rec[:].broadcast_to([128, 2, 2, DK]))
                for qc in range(4):
                    nc.sync.dma_start_transpose(
                        ctxT[j][:, qb * QB + qc * 128:
                                qb * QB + (qc + 1) * 128],
                        csw[:, qc, :, :])

            def alloc_cp():
                cp = [psum.tile([128, 2, 2, DK + 1], F32, tag=f"cp{i}",
                                name=f"cp{i}", bufs=1) for i in range(2)]
                for i in range(2):
                    nc.vector.memset(cp[i][:], 0.0)
                return cp

            # ---------------- pipeline ----------------
            nc.vector.memset(bias2[:], -2.0)
            load_w("q")
            load_x(0, ("q",))
            load_w("k")
            load_x(0, ("k",))
            load_const()
            load_w("v")
            load_x(0, ("v",), chunked=True)
            proj_qk_unit("q", 0, 0)
            proj_qk_unit("k", 0, 0)

            pending = []
            outproj_pending = []

            def drain(n=1):
                for _ in range(n):
                    if pending:
                        pending.pop(0)()

            # ---- qb0: scores-first (j-pairwise), v-units after xv lands
            e01s = {}
            load_x(1, ("q", "k"))
            for m in range(1, 4):
                pending.append(lambda m=m: proj_qk_unit("q", m, 0))
                pending.append(lambda m=m: proj_qk_unit("k", m, 0))
            for j in (0, 1):
                for t in range(4):
                    sc_exp(0, j, t, e01s)
                    drain()
            while pending:
                pending.pop(0)()
            for sc in range(4):
                proj_v_unit(sc, 0)
            load_x(1, ("v",))
            load_wo()
            pending.extend(proj_units(1))
            cp = alloc_cp()
            for t in range(4):
                ctx_unit(0, 0, t, e01s, cp)
            norm_unit(0, 0, cp)
            for t in range(4):
                sc_exp(0, 2, t, e01s)
            cp = alloc_cp()
            for t in range(4):
                ctx_unit(0, 1, t, e01s, cp)
                if t >= 2:
                    drain()
            norm_unit(0, 1, cp)
            for t in range(4):
                sc_exp(0, 3, t, e01s)
                drain()
            for j in (2, 3):
                cp = alloc_cp()
                for t in range(4):
                    ctx_unit(0, j, t, e01s, cp)
                    drain(2)
                norm_unit(0, j, cp)
            while pending:
                pending.pop(0)()
            for sc in range(4):
                outproj_pending.append(lambda sc=sc: outproj_unit(sc))

            # ---- qb1..3: streaming
            for qb in range(1, n_qb):
                if qb + 1 < n_ts:
                    load_x(qb + 1)
                    pending.extend(proj_units(qb + 1))
                if qb == n_qb - 1:
                    pending.extend(outproj_pending)
                    outproj_pending = []
                nt = (qb + 1) * (QB // KT)
                n_steps = 4 * nt
                len0 = len(pending)
                drain_by = n_steps
                step = 0
                popped = 0
                e01s = {}
                for j in range(4):
                    cp = alloc_cp()
                    sc_exp(qb, j, 0, e01s)
                    sc_exp(qb, j, 1, e01s)
                    for t in range(nt):
                        if t + 2 < nt:
                            sc_exp(qb, j, t + 2, e01s)
                        ctx_unit(qb, j, t, e01s, cp)
                        step += 1
                        if qb == n_qb - 1 or step > nt // 2:
                            target = min(len0, (step * len0) // drain_by)
                            while popped < target and pending:
                                pending.pop(0)()
                                popped += 1
                    norm_unit(qb, j, cp)
                while pending:
                    pending.pop(0)()
                for sc in range(qb * 4, (qb + 1) * 4):
                    outproj_pending.append(lambda sc=sc: outproj_unit(sc))
            pending = outproj_pending
            while pending:
                pending.pop(0)()
            psum.release()

    nc.compile()
    return nc


def _get_nc(s=S):
    if s not in _CACHE:
        _CACHE[s] = _build(s)
    return _CACHE[s]


def _make_masks(s=S):
    m = np.zeros((KT, KT), np.float32)
    for kk in range(KT):
        m[kk, kk:] = 1.0
    return m.astype(np.float16)


def _q8(x):
    return np.ascontiguousarray(x).astype(NF8)


def _split2(x):
    """x (f32) -> (hi, lo*32) fp8 arrays."""
    xh = _q8(x)
    xl = _q8(32.0 * (x - xh.astype(np.float32)))
    return xh, xl


def _wsplit3(W):
    """W slice (f32, [O, D]) -> transposed fp8 arrays (hi, hi/32, lo)
    at scale alpha=512."""
    W2 = np.ascontiguousarray(512.0 * W.T)      # [D, O]
    Wh = _q8(W2)
    W3 = _q8(Wh.astype(np.float32) / 32.0)
    Wl = _q8(W2 - Wh.astype(np.float32))
    return Wh, W3, Wl


def make_in_maps(q, k, v, Wq, bq, Wk, bk, Wv, bv, Wo, s=S):
    masks = _make_masks(s)
    ones8 = np.ones((128, HPC), np.float16)
    xs = {}
    for b in range(B):
        for nm, arr in (("q", q), ("k", k), ("v", v)):
            xT = np.ascontiguousarray(arr[b].T)
            xs[(b, nm)] = _split2(xT)
    in_maps = []
    for c in range(N_CORES):
        b, g = c // 2, c % 2
        gsl = slice(g * O, (g + 1) * O)
        m = {}
        for nm in ("q", "k", "v"):
            xh, xl = xs[(b, nm)]
            m[f"x{nm}h"], m[f"x{nm}l"] = xh, xl
        for nm, W in (("q", Wq), ("k", Wk), ("v", Wv)):
            Wh, W3, Wl = _wsplit3(W[gsl, :])
            m[f"w{nm}h"], m[f"w{nm}3"], m[f"w{nm}l"] = Wh, W3, Wl
        m["bq"] = np.ascontiguousarray(bq[gsl])
        m["bk"] = np.ascontiguousarray(bk[gsl])
        m["bv_bc"] = np.ascontiguousarray(
            np.broadcast_to(bv[gsl][None, :], (128, O)))
        m["woT"] = np.ascontiguousarray(Wo[:, gsl].T).astype(np.float16)
        m["masks"] = masks
        m["ones8"] = ones8
        in_maps.append(m)
    return in_maps


def kernel(q, k, v, mask, Wq, bq, Wk, bk, Wv, bv, Wo, bo):
    q = np.asarray(q, np.float32)
    k = np.asarray(k, np.float32)
    v = np.asarray(v, np.float32)
    nc = _get_nc(S)
    in_maps = make_in_maps(
        q, k, v,
        np.asarray(Wq, np.float32), np.asarray(bq, np.float32),
        np.asarray(Wk, np.float32), np.asarray(bk, np.float32),
        np.asarray(Wv, np.float32), np.asarray(bv, np.float32),
        np.asarray(Wo, np.float32), S)
    res = run_bass_kernel_spmd(nc, in_maps, list(range(N_CORES)))
    bo = np.asarray(bo, np.float32)
    out = np.empty((B, S, D), np.float32)
    for b in range(B):
        out[b] = (res.results[2 * b]["out"].astype(np.float32)
                  + res.results[2 * b + 1]["out"].astype(np.float32) + bo)
    return out


# revision 59
# speedup vs baseline: 1.0079x; 1.0079x over previous
"""Multi-head attention Trainium2 kernel (B=4, S=2048, D=1024, H=16, causal).

Sharding: 8 cores = 4 batches x 2 head-groups (8 heads each, tensor-parallel
over the QKV/out projection weights along the head dimension).

Per-core strategy (v2 — fp8 DoubleRow projections + f16 attention):
  - Projections run as fp8e4m3 DoubleRow matmuls (0.5 PE cycles/row, two
    128-deep k-chunks per instruction) with a 3-term error correction:
      W'x ~= Wh.xh + (Wh/32).(32 xl) + (W' - Wh).xh
    where W' = 512W (host-scaled so both Wh and the residual W'-Wh sit in
    e4m3's normal range), xh = q8(x), xl = x - xh.  All three terms share
    one PSUM accumulation (scale 512); the DVE bias-add folds the 1/512
    back in.  All quantization happens on the host, shipped as fp8 arrays
    (2 bytes/activation value vs 4 for f32 -> half the DMA).
  - Scores in f16 (fp8 scores fail the 2e-2 gate), exp on ACT with
    scale=1/8 and bias=-2 folded in (denominator self-normalizes), E in
    f16 (max logit 10.31 -> e^8.3 fits f16 comfortably).
  - ctx matmuls are operand-swapped: out [128 q, 65] (cost-model charges
    output free size only -> 65 rows instead of 512), accumulated over
    k-tiles in PSUM; V carries a ones column producing the denominator.
  - normalize on DVE (per-partition reciprocal scalars, no partition
    broadcast needed), then a DMA transpose re-orients ctx to d-major in
    ctxT for the out-projection.
  - Projections for s-block ts+1 and out-projections are injected as PE
    filler inside the ACT-paced attention of q-block qb=ts.
"""

import numpy as np
import ml_dtypes

import concourse.bacc as bacc
import concourse.mybir as mybir
import concourse.tile as tile
from concourse.bass_utils import run_bass_kernel_spmd

B, S, D, H = 4, 2048, 1024, 16
DK = D // H          # 64
N_CORES = 8
O = 512              # head dims per core (8 heads x 64)
HPC = 8              # heads per core
SB = 512             # s-block for projections
QB = 512             # q-block for attention
KT = 128             # k tile
F32 = mybir.dt.float32
F16 = mybir.dt.float16
F8 = mybir.dt.float8e4
DR = mybir.MatmulPerfMode.DoubleRow
NF8 = ml_dtypes.float8_e4m3

_CACHE = {}


def _build(s=S):
    nc = bacc.Bacc("TRN2", target_bir_lowering=False, debug=False,
                   num_devices=N_CORES)
    n_ts = s // SB
    n_qb = s // QB
    n_sc = s // 128

    # fp8 activation arrays [D, s]: hi, lo*32
    xd = {}
    for t in ("q", "k", "v"):
        for a in ("h", "l"):
            xd[t + a] = nc.declare_dram_parameter(f"x{t}{a}", [D, s], F8,
                                                  isOutput=False)
    # fp8 weight arrays [D, O]: hi, hi/32, lo*32
    wd = {}
    for t in ("q", "k", "v"):
        for a in ("h", "3", "l"):
            wd[t + a] = nc.declare_dram_parameter(f"w{t}{a}", [D, O], F8,
                                                  isOutput=False)
    bqd = nc.declare_dram_parameter("bq", [O], F32, isOutput=False)
    bkd = nc.declare_dram_parameter("bk", [O], F32, isOutput=False)
    bvb = nc.declare_dram_parameter("bv_bc", [128, O], F32, isOutput=False)
    wod = nc.declare_dram_parameter("woT", [O, D], F16, isOutput=False)
    maskd = nc.declare_dram_parameter("masks", [KT, KT], F16, isOutput=False)
    onesd = nc.declare_dram_parameter("ones8", [128, HPC], F16,
                                      isOutput=False)
    outd = nc.declare_dram_parameter("out", [s, D], F16, isOutput=True)

    with tile.TileContext(nc) as tc:
        with tc.tile_pool(name="res", bufs=1) as res, \
             tc.tile_pool(name="wpool", bufs=1) as wpool, \
             tc.tile_pool(name="xpool", bufs=2) as xpool, \
             tc.tile_pool(name="epool", bufs=8) as epool, \
             tc.tile_pool(name="npool", bufs=4) as npool, \
             tc.tile_pool(name="outpool", bufs=4) as outpool:
            psum = tc.alloc_tile_pool(name="psum", bufs=2, space="PSUM")

            qhT = [res.tile([128, s], F16, tag=f"qhT{j}", name=f"qhT{j}")
                   for j in range(4)]
            khT = [res.tile([128, s], F16, tag=f"khT{j}", name=f"khT{j}")
                   for j in range(4)]
            vh = [res.tile([128, HPC, DK + 1], F16, tag=f"vh{i}",
                           name=f"vh{i}") for i in range(n_sc)]
            ctxT = [res.tile([128, s], F16, tag=f"ctxT{j}", name=f"ctxT{j}")
                    for j in range(4)]
            bq_t = res.tile([128, O // 128], F32, tag="bq_t", name="bq_t")
            bk_t = res.tile([128, O // 128], F32, tag="bk_t", name="bk_t")
            bv_t = res.tile([128, O], F32, tag="bv_t", name="bv_t")
            masks = res.tile([128, KT], F16, tag="masks", name="masks")
            ones_t = res.tile([128, HPC], F16, tag="ones_t", name="ones_t")
            wo_sb = [res.tile([128, D], F16, tag=f"wo{jw}", name=f"wo{jw}")
                     for jw in range(4)]
            bias2 = res.tile([128, 1], F32, tag="bias2", name="bias2")

            # weight tiles [128, 8, O] fp8 (d-chunk as free dim)
            wt = {}
            for t in ("q", "k", "v"):
                for a in ("h", "3", "l"):
                    wt[t + a] = wpool.tile([128, 8, O], F8, tag=f"w{t}{a}",
                                           name=f"w{t}{a}")

            def load_w(t):
                for a in ("h", "3", "l"):
                    nc.sync.dma_start(
                        wt[t + a][:],
                        wd[t + a].ap().rearrange("(a p) m -> p a m",
                                                 p=128))

            def load_const():
                nc.sync.dma_start(
                    bq_t[:], bqd.ap().rearrange("(m p) -> p m", p=128))
                nc.sync.dma_start(
                    bk_t[:], bkd.ap().rearrange("(m p) -> p m", p=128))
                nc.sync.dma_start(bv_t[:], bvb[:, :])
                nc.sync.dma_start(masks[:], maskd[:, :])
                nc.sync.dma_start(ones_t[:], onesd[:, :])

            def load_wo():
                for jw in range(4):
                    nc.sync.dma_start(wo_sb[jw][:],
                                      wod[jw * 128:(jw + 1) * 128, :])

            xt = {}   # (ts, tensor, arr) -> tile

            def load_x(ts, tensors=("q", "k", "v"), chunked=False):
                for t in tensors:
                    for a in ("h", "l"):
                        tl = xpool.tile([128, 8, SB], F8, tag=f"x{t}{a}",
                                        name=f"x{t}{a}{ts}")
                        src_r = xd[t + a].ap().rearrange(
                            "(a p) s -> p a s", p=128)
                        eng = nc.sync
                        if chunked:
                            for c4 in range(4):
                                ssl = slice(ts * SB + c4 * 128,
                                            ts * SB + (c4 + 1) * 128)
                                eng.dma_start(
                                    tl[:, :, c4 * 128:(c4 + 1) * 128],
                                    src_r[:, :, ssl])
                        else:
                            ssl = slice(ts * SB, (ts + 1) * SB)
                            eng.dma_start(tl[:], src_r[:, :, ssl])
                        xt[(ts, t, a)] = tl

            def proj_qk_unit(t, m, ts, hf=None):
                """One m-tile of the q or k projection for s-block ts."""
                xh, xl = xt[(ts, t, "h")], xt[(ts, t, "l")]
                msl = slice(m * 128, (m + 1) * 128)
                ps = psum.tile([128, SB], F32, tag="fo", name=f"ps{t}")
                for c in range(4):
                    cs = slice(2 * c, 2 * c + 2)
                    nc.tensor.matmul(ps[:], wt[t + "h"][:, cs, msl],
                                     xh[:, cs, :], start=(c == 0),
                                     stop=False, perf_mode=DR)
                for c in range(4):
                    cs = slice(2 * c, 2 * c + 2)
                    nc.tensor.matmul(ps[:], wt[t + "3"][:, cs, msl],
                                     xl[:, cs, :], start=False, stop=False,
                                     perf_mode=DR)
                for c in range(4):
                    cs = slice(2 * c, 2 * c + 2)
                    nc.tensor.matmul(ps[:], wt[t + "l"][:, cs, msl],
                                     xh[:, cs, :], start=False,
                                     stop=(c == 3), perf_mode=DR)
                dstT = qhT if t == "q" else khT
                bias = bq_t if t == "q" else bk_t
                ssl = slice(ts * SB, (ts + 1) * SB)
                nc.vector.scalar_tensor_tensor(
                    dstT[m][:, ssl], ps[:], 1.0 / 512.0,
                    bias[:, m:m + 1].broadcast_to([128, SB]),
                    op0=mybir.AluOpType.mult, op1=mybir.AluOpType.add)

            def proj_v_unit(sc, ts):
                """One 128-row s-chunk of the v projection for s-block ts."""
                xh, xl = xt[(ts, "v", "h")], xt[(ts, "v", "l")]
                scl = slice(sc * 128, (sc + 1) * 128)
                si = ts * (SB // 128) + sc
                ps = psum.tile([128, O], F32, tag="fo", name="psv")
                for c in range(4):
                    cs = slice(2 * c, 2 * c + 2)
                    nc.tensor.matmul(ps[:], xh[:, cs, scl],
                                     wt["vh"][:, cs, :], start=(c == 0),
                                     stop=False, perf_mode=DR)
                for c in range(4):
                    cs = slice(2 * c, 2 * c + 2)
                    nc.tensor.matmul(ps[:], xl[:, cs, scl],
                                     wt["v3"][:, cs, :], start=False,
                                     stop=False, perf_mode=DR)
                for c in range(4):
                    cs = slice(2 * c, 2 * c + 2)
                    nc.tensor.matmul(ps[:], xh[:, cs, scl],
                                     wt["vl"][:, cs, :], start=False,
                                     stop=(c == 3), perf_mode=DR)
                nc.vector.scalar_tensor_tensor(
                    vh[si][:, :, 0:DK],
                    ps[:].rearrange("p (h e) -> p h e", e=DK), 1.0 / 512.0,
                    bv_t[:].rearrange("p (h e) -> p h e", e=DK),
                    op0=mybir.AluOpType.mult, op1=mybir.AluOpType.add)
                nc.vector.tensor_copy(vh[si][:, :, DK], ones_t[:])

            def proj_units(ts):
                units = []
                for t in ("q", "k"):
                    for m in range(4):
                        units.append(lambda t=t, m=m, ts=ts:
                                     proj_qk_unit(t, m, ts))
                for sc in range(SB // 128):
                    units.append(lambda sc=sc, ts=ts: proj_v_unit(sc, ts))
                return units

            def outproj_unit(sc):
                ot = outpool.tile([128, D], F16, tag="out_t", name="ot")
                for oc in range(2):
                    osl = slice(oc * 512, (oc + 1) * 512)
                    po = psum.tile([128, 512], F32, tag="fo", name="po")
                    for jw in range(4):
                        nc.tensor.matmul(
                            po[:], ctxT[jw][:, sc * 128:(sc + 1) * 128],
                            wo_sb[jw][:, osl], start=(jw == 0),
                            stop=(jw == 3))
                    nc.vector.tensor_copy(ot[:, osl], po[:])
                nc.sync.dma_start(outd[sc * 128:(sc + 1) * 128, :], ot[:])


            def sc_exp(qb, j, t, e01s):
                nt = (qb + 1) * (QB // KT)
                ksl = slice(t * KT, (t + 1) * KT)
                jj = t - (QB // KT) * qb
                lo = jj * KT if jj > 0 else 0
                qn = slice(qb * QB + lo, (qb + 1) * QB)
                s01 = psum.tile([128, 2, QB], F32, tag="sc", name="s01")
                nc.tensor.matmul(
                    s01[:, 0, lo:], khT[j][0:64, ksl],
                    qhT[j][0:64, qn], start=True, stop=True)
                nc.tensor.matmul(
                    s01[:, 1, lo:], khT[j][64:128, ksl],
                    qhT[j][64:128, qn], start=True, stop=True,
                    tile_position=(64, 0))
                e01 = epool.tile([128, 2, QB], F16, tag="e01", name="e01")
                nc.scalar.activation(
                    e01[:, :, lo:], s01[:, :, lo:],
                    mybir.ActivationFunctionType.Exp,
                    bias=bias2[:], scale=0.125)
                if jj >= 0:
                    nc.vector.tensor_mul(
                        e01[:, :, lo:lo + KT],
                        e01[:, :, lo:lo + KT],
                        masks[:].unsqueeze(1).broadcast_to([128, 2, KT]))
                e01s[(j, t)] = e01

            def ctx_unit(qb, j, t, e01s, cp):
                jj = t - (QB // KT) * qb
                e01 = e01s[(j, t)]
                for qc in range(max(jj, 0), 4):
                    last = 4 * qb + qc
                    qcs = slice(qc * KT, (qc + 1) * KT)
                    for h in range(2):
                        nc.tensor.matmul(
                            cp[qc // 2][:, qc % 2, h, :],
                            e01[:, h, qcs], vh[t][:, 2 * j + h, :],
                            start=False, stop=False,
                            skip_group_check=True)

            def norm_unit(qb, j, cp):
                csw = npool.tile([128, 4, 2, DK], F16, tag="csw",
                                 name="csw")
                for i in range(2):
                    rec = npool.tile([128, 2, 2, 1], F32, tag=f"rec{i}",
                                     name=f"rec{i}")
                    nc.vector.reciprocal(rec[:],
                                         cp[i][:, :, :, DK:DK + 1])
                    nc.vector.tensor_mul(
                        csw[:, 2 * i:2 * i + 2, :, :],
                        cp[i][:, :, :, 0:DK],
                        rec[:].broadcast_to([128, 2, 2, DK]))
                for qc in range(4):
                    nc.sync.dma_start_transpose(
                        ctxT[j][:, qb * QB + qc * 128:
                                qb * QB + (qc + 1) * 128],
                        csw[:, qc, :, :])

            def alloc_cp():
                cp = [psum.tile([128, 2, 2, DK + 1], F32, tag=f"cp{i}",
                                name=f"cp{i}", bufs=1) for i in range(2)]
                for i in range(2):
                    nc.vector.memset(cp[i][:], 0.0)
                return cp

            # ---------------- pipeline ----------------
            nc.vector.memset(bias2[:], -2.0)
            load_w("q")
            load_x(0, ("q",))
            load_w("k")
            load_x(0, ("k",))
            load_const()
            load_w("v")
            load_x(0, ("v",), chunked=True)
            proj_qk_unit("q", 0, 0)
            proj_qk_unit("k", 0, 0)

            pending = []
            outproj_pending = []

            def drain(n=1):
                for _ in range(n):
                    if pending:
                        pending.pop(0)()

            # ---- qb0: scores-first (j-pairwise), v-units after xv lands
            e01s = {}
            load_x(1, ("q", "k"))
            for m in range(1, 4):
                pending.append(lambda m=m: proj_qk_unit("q", m, 0))
                pending.append(lambda m=m: proj_qk_unit("k", m, 0))
            for j in (0, 1):
                for t in range(4):
                    sc_exp(0, j, t, e01s)
                    drain()
            while pending:
                pending.pop(0)()
            for sc in range(4):
                proj_v_unit(sc, 0)
            load_x(1, ("v",))
            load_wo()
            pending.extend(proj_units(1))
            cp = alloc_cp()
            for t in range(4):
                ctx_unit(0, 0, t, e01s, cp)
            norm_unit(0, 0, cp)
            for t in range(4):
                sc_exp(0, 2, t, e01s)
            cp = alloc_cp()
            for t in range(4):
                ctx_unit(0, 1, t, e01s, cp)
                if t >= 2:
                    drain()
            norm_unit(0, 1, cp)
            for t in range(4):
                sc_exp(0, 3, t, e01s)
                drain()
            for j in (2, 3):
                cp = alloc_cp()
                for t in range(4):
                    ctx_unit(0, j, t, e01s, cp)
                    drain(2)
                norm_unit(0, j, cp)
            while pending:
                pending.pop(0)()
            for sc in range(4):
                outproj_pending.append(lambda sc=sc: outproj_unit(sc))

            # ---- qb1..3: streaming
            for qb in range(1, n_qb):
                if qb + 1 < n_ts:
                    load_x(qb + 1)
                    pending.extend(proj_units(qb + 1))
                if qb == n_qb - 1:
                    pending.extend(outproj_pending)
                    outproj_pending = []
                nt = (qb + 1) * (QB // KT)
                n_steps = 4 * nt
                len0 = len(pending)
                drain_by = n_steps
                step = 0
                popped = 0
                e01s = {}
                for j in range(4):
                    cp = alloc_cp()
                    sc_exp(qb, j, 0, e01s)
                    sc_exp(qb, j, 1, e01s)
                    sc_exp(qb, j, 2, e01s)
                    sc_exp(qb, j, 3, e01s)
                    for t in range(nt):
                        if t + 4 < nt:
                            sc_exp(qb, j, t + 4, e01s)
                        ctx_unit(qb, j, t, e01s, cp)
                        step += 1
                        if qb == n_qb - 1 or step > nt // 2:
                            target = min(len0, (step * len0) // drain_by)
                            while popped < target and pending:
                                pending.pop(0)()
                                popped += 1
                    norm_unit(qb, j, cp)
                while pending:
                    pending.pop(0)()
                for sc in range(qb * 4, (qb + 1) * 4):
                    outproj_pending.append(lambda sc=sc: outproj_unit(sc))
            pending = outproj_pending
            while pending:
                pending.pop(0)()
            psum.release()

    nc.compile()
    return nc


def _get_nc(s=S):
    if s not in _CACHE:
        _CACHE[s] = _build(s)
    return _CACHE[s]


def _make_masks(s=S):
    m = np.zeros((KT, KT), np.float32)
    for kk in range(KT):
        m[kk, kk:] = 1.0
    return m.astype(np.float16)


def _q8(x):
    return np.ascontiguousarray(x).astype(NF8)


def _split2(x):
    """x (f32) -> (hi, lo*32) fp8 arrays."""
    xh = _q8(x)
    xl = _q8(32.0 * (x - xh.astype(np.float32)))
    return xh, xl


def _wsplit3(W):
    """W slice (f32, [O, D]) -> transposed fp8 arrays (hi, hi/32, lo)
    at scale alpha=512."""
    W2 = np.ascontiguousarray(512.0 * W.T)      # [D, O]
    Wh = _q8(W2)
    W3 = _q8(Wh.astype(np.float32) / 32.0)
    Wl = _q8(W2 - Wh.astype(np.float32))
    return Wh, W3, Wl


def make_in_maps(q, k, v, Wq, bq, Wk, bk, Wv, bv, Wo, s=S):
    masks = _make_masks(s)
    ones8 = np.ones((128, HPC), np.float16)
    xs = {}
    for b in range(B):
        for nm, arr in (("q", q), ("k", k), ("v", v)):
            xT = np.ascontiguousarray(arr[b].T)
            xs[(b, nm)] = _split2(xT)
    in_maps = []
    for c in range(N_CORES):
        b, g = c // 2, c % 2
        gsl = slice(g * O, (g + 1) * O)
        m = {}
        for nm in ("q", "k", "v"):
            xh, xl = xs[(b, nm)]
            m[f"x{nm}h"], m[f"x{nm}l"] = xh, xl
        for nm, W in (("q", Wq), ("k", Wk), ("v", Wv)):
            Wh, W3, Wl = _wsplit3(W[gsl, :])
            m[f"w{nm}h"], m[f"w{nm}3"], m[f"w{nm}l"] = Wh, W3, Wl
        m["bq"] = np.ascontiguousarray(bq[gsl])
        m["bk"] = np.ascontiguousarray(bk[gsl])
        m["bv_bc"] = np.ascontiguousarray(
            np.broadcast_to(bv[gsl][None, :], (128, O)))
        m["woT"] = np.ascontiguousarray(Wo[:, gsl].T).astype(np.float16)
        m["masks"] = masks
        m["ones8"] = ones8
        in_maps.append(m)
    return in_maps


def kernel(q, k, v, mask, Wq, bq, Wk, bk, Wv, bv, Wo, bo):
    q = np.asarray(q, np.float32)
    k = np.asarray(k, np.float32)
    v = np.asarray(v, np.float32)
    nc = _get_nc(S)
    in_maps = make_in_maps(
        q, k, v,
        np.asarray(Wq, np.float32), np.asarray(bq, np.float32),
        np.asarray(Wk, np.float32), np.asarray(bk, np.float32),
        np.asarray(Wv, np.float32), np.asarray(bv, np.float32),
        np.asarray(Wo, np.float32), S)
    res = run_bass_kernel_spmd(nc, in_maps, list(range(N_CORES)))
    bo = np.asarray(bo, np.float32)
    out = np.empty((B, S, D), np.float32)
    for b in range(B):
        out[b] = (res.results[2 * b]["out"].astype(np.float32)
                  + res.results[2 * b + 1]["out"].astype(np.float32) + bo)
    return out


# revision 64
# speedup vs baseline: 1.0096x; 1.0017x over previous
"""Multi-head attention Trainium2 kernel (B=4, S=2048, D=1024, H=16, causal).

Sharding: 8 cores = 4 batches x 2 head-groups (8 heads each, tensor-parallel
over the QKV/out projection weights along the head dimension).

Per-core strategy (v2 — fp8 DoubleRow projections + f16 attention):
  - Projections run as fp8e4m3 DoubleRow matmuls (0.5 PE cycles/row, two
    128-deep k-chunks per instruction) with a 3-term error correction:
      W'x ~= Wh.xh + (Wh/32).(32 xl) + (W' - Wh).xh
    where W' = 512W (host-scaled so both Wh and the residual W'-Wh sit in
    e4m3's normal range), xh = q8(x), xl = x - xh.  All three terms share
    one PSUM accumulation (scale 512); the DVE bias-add folds the 1/512
    back in.  All quantization happens on the host, shipped as fp8 arrays
    (2 bytes/activation value vs 4 for f32 -> half the DMA).
  - Scores in f16 (fp8 scores fail the 2e-2 gate), exp on ACT with
    scale=1/8 and bias=-2 folded in (denominator self-normalizes), E in
    f16 (max logit 10.31 -> e^8.3 fits f16 comfortably).
  - ctx matmuls are operand-swapped: out [128 q, 65] (cost-model charges
    output free size only -> 65 rows instead of 512), accumulated over
    k-tiles in PSUM; V carries a ones column producing the denominator.
  - normalize on DVE (per-partition reciprocal scalars, no partition
    broadcast needed), then a DMA transpose re-orients ctx to d-major in
    ctxT for the out-projection.
  - Projections for s-block ts+1 and out-projections are injected as PE
    filler inside the ACT-paced attention of q-block qb=ts.
"""

import numpy as np
import ml_dtypes

import concourse.bacc as bacc
import concourse.mybir as mybir
import concourse.tile as tile
from concourse.bass_utils import run_bass_kernel_spmd

B, S, D, H = 4, 2048, 1024, 16
DK = D // H          # 64
N_CORES = 8
O = 512              # head dims per core (8 heads x 64)
HPC = 8              # heads per core
SB = 512             # s-block for projections
QB = 512             # q-block for attention
KT = 128             # k tile
F32 = mybir.dt.float32
F16 = mybir.dt.float16
F8 = mybir.dt.float8e4
DR = mybir.MatmulPerfMode.DoubleRow
NF8 = ml_dtypes.float8_e4m3

_CACHE = {}


def _build(s=S):
    nc = bacc.Bacc("TRN2", target_bir_lowering=False, debug=False,
                   num_devices=N_CORES)
    n_ts = s // SB
    n_qb = s // QB
    n_sc = s // 128

    # fp8 activation arrays [D, s]: hi, lo*32
    xd = {}
    for t in ("q", "k", "v"):
        for a in ("h", "l"):
            xd[t + a] = nc.declare_dram_parameter(f"x{t}{a}", [D, s], F8,
                                                  isOutput=False)
    # fp8 weight arrays [D, O]: hi, hi/32, lo*32
    wd = {}
    for t in ("q", "k", "v"):
        for a in ("h", "3", "l"):
            wd[t + a] = nc.declare_dram_parameter(f"w{t}{a}", [D, O], F8,
                                                  isOutput=False)
    bqd = nc.declare_dram_parameter("bq", [O], F32, isOutput=False)
    bkd = nc.declare_dram_parameter("bk", [O], F32, isOutput=False)
    bvb = nc.declare_dram_parameter("bv_bc", [128, O], F32, isOutput=False)
    wod = nc.declare_dram_parameter("woT", [O, D], F16, isOutput=False)
    maskd = nc.declare_dram_parameter("masks", [KT, KT], F16, isOutput=False)
    onesd = nc.declare_dram_parameter("ones8", [128, HPC], F16,
                                      isOutput=False)
    outd = nc.declare_dram_parameter("out", [s, D], F16, isOutput=True)

    with tile.TileContext(nc) as tc:
        with tc.tile_pool(name="res", bufs=1) as res, \
             tc.tile_pool(name="wpool", bufs=1) as wpool, \
             tc.tile_pool(name="xpool", bufs=2) as xpool, \
             tc.tile_pool(name="epool", bufs=8) as epool, \
             tc.tile_pool(name="npool", bufs=4) as npool, \
             tc.tile_pool(name="outpool", bufs=4) as outpool:
            psum = tc.alloc_tile_pool(name="psum", bufs=2, space="PSUM")

            qhT = [res.tile([128, s], F16, tag=f"qhT{j}", name=f"qhT{j}")
                   for j in range(4)]
            khT = [res.tile([128, s], F16, tag=f"khT{j}", name=f"khT{j}")
                   for j in range(4)]
            vh = [res.tile([128, HPC, DK + 1], F16, tag=f"vh{i}",
                           name=f"vh{i}") for i in range(n_sc)]
            ctxT = [res.tile([128, s], F16, tag=f"ctxT{j}", name=f"ctxT{j}")
                    for j in range(4)]
            bq_t = res.tile([128, O // 128], F32, tag="bq_t", name="bq_t")
            bk_t = res.tile([128, O // 128], F32, tag="bk_t", name="bk_t")
            bv_t = res.tile([128, O], F32, tag="bv_t", name="bv_t")
            masks = res.tile([128, KT], F16, tag="masks", name="masks")
            ones_t = res.tile([128, HPC], F16, tag="ones_t", name="ones_t")
            wo_sb = [res.tile([128, D], F16, tag=f"wo{jw}", name=f"wo{jw}")
                     for jw in range(4)]
            bias2 = res.tile([128, 1], F32, tag="bias2", name="bias2")

            # weight tiles [128, 8, O] fp8 (d-chunk as free dim)
            wt = {}
            for t in ("q", "k", "v"):
                for a in ("h", "3", "l"):
                    wt[t + a] = wpool.tile([128, 8, O], F8, tag=f"w{t}{a}",
                                           name=f"w{t}{a}")

            def load_w(t):
                for a in ("h", "3", "l"):
                    nc.sync.dma_start(
                        wt[t + a][:],
                        wd[t + a].ap().rearrange("(a p) m -> p a m",
                                                 p=128))

            def load_const():
                nc.sync.dma_start(
                    bq_t[:], bqd.ap().rearrange("(m p) -> p m", p=128))
                nc.sync.dma_start(
                    bk_t[:], bkd.ap().rearrange("(m p) -> p m", p=128))
                nc.sync.dma_start(bv_t[:], bvb[:, :])
                nc.sync.dma_start(masks[:], maskd[:, :])
                nc.sync.dma_start(ones_t[:], onesd[:, :])

            def load_wo():
                for jw in range(4):
                    nc.sync.dma_start(wo_sb[jw][:],
                                      wod[jw * 128:(jw + 1) * 128, :])

            xt = {}   # (ts, tensor, arr) -> tile

            def load_x(ts, tensors=("q", "k", "v"), chunked=False):
                for t in tensors:
                    for a in ("h", "l"):
                        tl = xpool.tile([128, 8, SB], F8, tag=f"x{t}{a}",
                                        name=f"x{t}{a}{ts}")
                        src_r = xd[t + a].ap().rearrange(
                            "(a p) s -> p a s", p=128)
                        eng = nc.sync
                        if chunked:
                            for c4 in range(4):
                                ssl = slice(ts * SB + c4 * 128,
                                            ts * SB + (c4 + 1) * 128)
                                eng.dma_start(
                                    tl[:, :, c4 * 128:(c4 + 1) * 128],
                                    src_r[:, :, ssl])
                        else:
                            ssl = slice(ts * SB, (ts + 1) * SB)
                            eng.dma_start(tl[:], src_r[:, :, ssl])
                        xt[(ts, t, a)] = tl

            def proj_qk_unit(t, m, ts, hf=None):
                """One m-tile of the q or k projection for s-block ts."""
                xh, xl = xt[(ts, t, "h")], xt[(ts, t, "l")]
                msl = slice(m * 128, (m + 1) * 128)
                ps = psum.tile([128, SB], F32, tag="fo", name=f"ps{t}")
                for c in range(4):
                    cs = slice(2 * c, 2 * c + 2)
                    nc.tensor.matmul(ps[:], wt[t + "h"][:, cs, msl],
                                     xh[:, cs, :], start=(c == 0),
                                     stop=False, perf_mode=DR)
                for c in range(4):
                    cs = slice(2 * c, 2 * c + 2)
                    nc.tensor.matmul(ps[:], wt[t + "3"][:, cs, msl],
                                     xl[:, cs, :], start=False, stop=False,
                                     perf_mode=DR)
                for c in range(4):
                    cs = slice(2 * c, 2 * c + 2)
                    nc.tensor.matmul(ps[:], wt[t + "l"][:, cs, msl],
                                     xh[:, cs, :], start=False,
                                     stop=(c == 3), perf_mode=DR)
                dstT = qhT if t == "q" else khT
                bias = bq_t if t == "q" else bk_t
                ssl = slice(ts * SB, (ts + 1) * SB)
                nc.vector.scalar_tensor_tensor(
                    dstT[m][:, ssl], ps[:], 1.0 / 512.0,
                    bias[:, m:m + 1].broadcast_to([128, SB]),
                    op0=mybir.AluOpType.mult, op1=mybir.AluOpType.add)

            def proj_v_unit(sc, ts):
                """One 128-row s-chunk of the v projection for s-block ts."""
                xh, xl = xt[(ts, "v", "h")], xt[(ts, "v", "l")]
                scl = slice(sc * 128, (sc + 1) * 128)
                si = ts * (SB // 128) + sc
                ps = psum.tile([128, O], F32, tag="fo", name="psv")
                for c in range(4):
                    cs = slice(2 * c, 2 * c + 2)
                    nc.tensor.matmul(ps[:], xh[:, cs, scl],
                                     wt["vh"][:, cs, :], start=(c == 0),
                                     stop=False, perf_mode=DR)
                for c in range(4):
                    cs = slice(2 * c, 2 * c + 2)
                    nc.tensor.matmul(ps[:], xl[:, cs, scl],
                                     wt["v3"][:, cs, :], start=False,
                                     stop=False, perf_mode=DR)
                for c in range(4):
                    cs = slice(2 * c, 2 * c + 2)
                    nc.tensor.matmul(ps[:], xh[:, cs, scl],
                                     wt["vl"][:, cs, :], start=False,
                                     stop=(c == 3), perf_mode=DR)
                nc.vector.scalar_tensor_tensor(
                    vh[si][:, :, 0:DK],
                    ps[:].rearrange("p (h e) -> p h e", e=DK), 1.0 / 512.0,
                    bv_t[:].rearrange("p (h e) -> p h e", e=DK),
                    op0=mybir.AluOpType.mult, op1=mybir.AluOpType.add)
                nc.vector.tensor_copy(vh[si][:, :, DK], ones_t[:])

            def proj_units(ts):
                units = []
                for t in ("q", "k"):
                    for m in range(4):
                        units.append(lambda t=t, m=m, ts=ts:
                                     proj_qk_unit(t, m, ts))
                for sc in range(SB // 128):
                    units.append(lambda sc=sc, ts=ts: proj_v_unit(sc, ts))
                return units

            def outproj_unit(sc):
                ot = outpool.tile([128, D], F16, tag="out_t", name="ot")
                for oc in range(2):
                    osl = slice(oc * 512, (oc + 1) * 512)
                    po = psum.tile([128, 512], F32, tag="fo", name="po")
                    for jw in range(4):
                        nc.tensor.matmul(
                            po[:], ctxT[jw][:, sc * 128:(sc + 1) * 128],
                            wo_sb[jw][:, osl], start=(jw == 0),
                            stop=(jw == 3))
                    nc.vector.tensor_copy(ot[:, osl], po[:])
                    nc.sync.dma_start(
                        outd[sc * 128:(sc + 1) * 128, osl], ot[:, osl])


            def sc_exp(qb, j, t, e01s):
                nt = (qb + 1) * (QB // KT)
                ksl = slice(t * KT, (t + 1) * KT)
                jj = t - (QB // KT) * qb
                lo = jj * KT if jj > 0 else 0
                qn = slice(qb * QB + lo, (qb + 1) * QB)
                s01 = psum.tile([128, 2, QB], F32, tag="sc", name="s01")
                nc.tensor.matmul(
                    s01[:, 0, lo:], khT[j][0:64, ksl],
                    qhT[j][0:64, qn], start=True, stop=True)
                nc.tensor.matmul(
                    s01[:, 1, lo:], khT[j][64:128, ksl],
                    qhT[j][64:128, qn], start=True, stop=True,
                    tile_position=(64, 0))
                e01 = epool.tile([128, 2, QB], F16, tag="e01", name="e01")
                nc.scalar.activation(
                    e01[:, :, lo:], s01[:, :, lo:],
                    mybir.ActivationFunctionType.Exp,
                    bias=bias2[:], scale=0.125)
                if jj >= 0:
                    nc.vector.tensor_mul(
                        e01[:, :, lo:lo + KT],
                        e01[:, :, lo:lo + KT],
                        masks[:].unsqueeze(1).broadcast_to([128, 2, KT]))
                e01s[(j, t)] = e01

            def ctx_unit(qb, j, t, e01s, cp):
                jj = t - (QB // KT) * qb
                e01 = e01s[(j, t)]
                for qc in range(max(jj, 0), 4):
                    last = 4 * qb + qc
                    qcs = slice(qc * KT, (qc + 1) * KT)
                    for h in range(2):
                        nc.tensor.matmul(
                            cp[qc // 2][:, qc % 2, h, :],
                            e01[:, h, qcs], vh[t][:, 2 * j + h, :],
                            start=False, stop=False,
                            skip_group_check=True)

            def norm_unit(qb, j, cp):
                csw = npool.tile([128, 4, 2, DK], F16, tag="csw",
                                 name="csw")
                for i in range(2):
                    rec = npool.tile([128, 2, 2, 1], F32, tag=f"rec{i}",
                                     name=f"rec{i}")
                    nc.vector.reciprocal(rec[:],
                                         cp[i][:, :, :, DK:DK + 1])
                    nc.vector.tensor_mul(
                        csw[:, 2 * i:2 * i + 2, :, :],
                        cp[i][:, :, :, 0:DK],
                        rec[:].broadcast_to([128, 2, 2, DK]))
                for qc in range(4):
                    nc.sync.dma_start_transpose(
                        ctxT[j][:, qb * QB + qc * 128:
                                qb * QB + (qc + 1) * 128],
                        csw[:, qc, :, :])

            def alloc_cp():
                cp = [psum.tile([128, 2, 2, DK + 1], F32, tag=f"cp{i}",
                                name=f"cp{i}", bufs=1) for i in range(2)]
                for i in range(2):
                    nc.vector.memset(cp[i][:], 0.0)
                return cp

            # ---------------- pipeline ----------------
            nc.vector.memset(bias2[:], -2.0)
            load_w("q")
            load_x(0, ("q",))
            load_w("k")
            load_x(0, ("k",))
            load_const()
            load_w("v")
            load_x(0, ("v",), chunked=True)
            proj_qk_unit("q", 0, 0)
            proj_qk_unit("k", 0, 0)

            pending = []
            outproj_pending = []

            def drain(n=1):
                for _ in range(n):
                    if pending:
                        pending.pop(0)()

            # ---- qb0: scores-first (j-pairwise), v-units after xv lands
            e01s = {}
            load_x(1, ("q", "k"))
            for m in range(1, 4):
                pending.append(lambda m=m: proj_qk_unit("q", m, 0))
                pending.append(lambda m=m: proj_qk_unit("k", m, 0))
            for j in (0, 1):
                for t in range(4):
                    sc_exp(0, j, t, e01s)
                    drain()
            while pending:
                pending.pop(0)()
            for sc in range(4):
                proj_v_unit(sc, 0)
            load_x(1, ("v",))
            load_wo()
            pending.extend(proj_units(1))
            cp = alloc_cp()
            for t in range(4):
                ctx_unit(0, 0, t, e01s, cp)
            norm_unit(0, 0, cp)
            for t in range(4):
                sc_exp(0, 2, t, e01s)
            cp = alloc_cp()
            for t in range(4):
                ctx_unit(0, 1, t, e01s, cp)
                if t >= 2:
                    drain()
            norm_unit(0, 1, cp)
            for t in range(4):
                sc_exp(0, 3, t, e01s)
                drain()
            for j in (2, 3):
                cp = alloc_cp()
                for t in range(4):
                    ctx_unit(0, j, t, e01s, cp)
                    drain(2)
                norm_unit(0, j, cp)
            while pending:
                pending.pop(0)()
            for sc in range(4):
                outproj_pending.append(lambda sc=sc: outproj_unit(sc))

            # ---- qb1..3: streaming
            for qb in range(1, n_qb):
                if qb + 1 < n_ts:
                    load_x(qb + 1)
                    pending.extend(proj_units(qb + 1))
                if qb == n_qb - 1:
                    pending.extend(outproj_pending)
                    outproj_pending = []
                nt = (qb + 1) * (QB // KT)
                n_steps = 4 * nt
                len0 = len(pending)
                drain_by = n_steps
                step = 0
                popped = 0
                e01s = {}
                for j in range(4):
                    cp = alloc_cp()
                    sc_exp(qb, j, 0, e01s)
                    sc_exp(qb, j, 1, e01s)
                    sc_exp(qb, j, 2, e01s)
                    sc_exp(qb, j, 3, e01s)
                    for t in range(nt):
                        if t + 4 < nt:
                            sc_exp(qb, j, t + 4, e01s)
                        ctx_unit(qb, j, t, e01s, cp)
                        step += 1
                        if qb == n_qb - 1 or step > nt // 2:
                            target = min(len0, (step * len0) // drain_by)
                            while popped < target and pending:
                                pending.pop(0)()
                                popped += 1
                    norm_unit(qb, j, cp)
                while pending:
                    pending.pop(0)()
                for sc in range(qb * 4, (qb + 1) * 4):
                    outproj_pending.append(lambda sc=sc: outproj_unit(sc))
            pending = outproj_pending
            while pending:
                pending.pop(0)()
            psum.release()

    nc.compile()
    return nc


def _get_nc(s=S):
    if s not in _CACHE:
        _CACHE[s] = _build(s)
    return _CACHE[s]


def _make_masks(s=S):
    m = np.zeros((KT, KT), np.float32)
    for kk in range(KT):
        m[kk, kk:] = 1.0
    return m.astype(np.float16)


def _q8(x):
    return np.ascontiguousarray(x).astype(NF8)


def _split2(x):
    """x (f32) -> (hi, lo*32) fp8 arrays."""
    xh = _q8(x)
    xl = _q8(32.0 * (x - xh.astype(np.float32)))
    return xh, xl


def _wsplit3(W):
    """W slice (f32, [O, D]) -> transposed fp8 arrays (hi, hi/32, lo)
    at scale alpha=512."""
    W2 = np.ascontiguousarray(512.0 * W.T)      # [D, O]
    Wh = _q8(W2)
    W3 = _q8(Wh.astype(np.float32) / 32.0)
    Wl = _q8(W2 - Wh.astype(np.float32))
    return Wh, W3, Wl


def make_in_maps(q, k, v, Wq, bq, Wk, bk, Wv, bv, Wo, s=S):
    masks = _make_masks(s)
    ones8 = np.ones((128, HPC), np.float16)
    xs = {}
    for b in range(B):
        for nm, arr in (("q", q), ("k", k), ("v", v)):
            xT = np.ascontiguousarray(arr[b].T)
            xs[(b, nm)] = _split2(xT)
    in_maps = []
    for c in range(N_CORES):
        b, g = c // 2, c % 2
        gsl = slice(g * O, (g + 1) * O)
        m = {}
        for nm in ("q", "k", "v"):
            xh, xl = xs[(b, nm)]
            m[f"x{nm}h"], m[f"x{nm}l"] = xh, xl
        for nm, W in (("q", Wq), ("k", Wk), ("v", Wv)):
            Wh, W3, Wl = _wsplit3(W[gsl, :])
            m[f"w{nm}h"], m[f"w{nm}3"], m[f"w{nm}l"] = Wh, W3, Wl
        m["bq"] = np.ascontiguousarray(bq[gsl])
        m["bk"] = np.ascontiguousarray(bk[gsl])
        m["bv_bc"] = np.ascontiguousarray(
            np.broadcast_to(bv[gsl][None, :], (128, O)))
        m["woT"] = np.ascontiguousarray(Wo[:, gsl].T).astype(np.float16)
        m["masks"] = masks
        m["ones8"] = ones8
        in_maps.append(m)
    return in_maps


def kernel(q, k, v, mask, Wq, bq, Wk, bk, Wv, bv, Wo, bo):
    q = np.asarray(q, np.float32)
    k = np.asarray(k, np.float32)
    v = np.asarray(v, np.float32)
    nc = _get_nc(S)
    in_maps = make_in_maps(
        q, k, v,
        np.asarray(Wq, np.float32), np.asarray(bq, np.float32),
        np.asarray(Wk, np.float32), np.asarray(bk, np.float32),
        np.asarray(Wv, np.float32), np.asarray(bv, np.float32),
        np.asarray(Wo, np.float32), S)
    res = run_bass_kernel_spmd(nc, in_maps, list(range(N_CORES)))
    bo = np.asarray(bo, np.float32)
    out = np.empty((B, S, D), np.float32)
    for b in range(B):
        out[b] = (res.results[2 * b]["out"].astype(np.float32)
                  + res.results[2 * b + 1]["out"].astype(np.float32) + bo)
    return out
